# revision 1
# baseline (speedup 1.0000x reference)
"""Trainium2 Bass kernel for the DTW mask calculator.

Computes, for N=8192, fp32:
    out = where(sd < 5, exp(-sd^2), 0) * where(labels[i]==labels[j], 1, 0.1)
          * exp(-dtw^2)
        = (sd < 5) * exp(-(sd^2 + dtw^2)) * max(labels[i]==labels[j], 0.1)

Row-sharded across 8 NeuronCores (1024 rows each). adj_mx is unused by the
reference computation and never uploaded. Per [128, 2048] chunk:
  ACT: z1 = Square(sd); z2 = Square(dtw); e = Exp(-(z1+z2))
  DVE: s = z1+z2; aext = max(lcol==lrow, 0.1) [one dual-op tensor_scalar];
       me = (sd<5)*e [one fused scalar_tensor_tensor]; out = me*aext
"""

import numpy as np

N = 8192
N_CORES = 8
R = N // N_CORES          # rows per core = 1024
P = 128                   # partitions
RT = R // P               # row tiles per core = 8
W = 2048                  # column chunk width
CT = N // W               # column chunks = 4

_CACHE = {}


def _build():
    import concourse.tile as tile
    from concourse import bacc, mybir

    f32 = mybir.dt.float32
    AF = mybir.ActivationFunctionType
    OP = mybir.AluOpType

    nc = bacc.Bacc("TRN2", target_bir_lowering=False, debug=False,
                   num_devices=N_CORES)

    sd = nc.dram_tensor("sd", [R, N], f32, kind="ExternalInput").ap()
    dtw = nc.dram_tensor("dtw", [R, N], f32, kind="ExternalInput").ap()
    lcol = nc.dram_tensor("lcol", [P, N], f32, kind="ExternalInput").ap()
    lrow = nc.dram_tensor("lrow", [P, RT], f32, kind="ExternalInput").ap()
    out = nc.dram_tensor("out", [R, N], f32, kind="ExternalOutput").ap()

    with tile.TileContext(nc) as tc:
        with (
            tc.tile_pool(name="const", bufs=1) as const,
            tc.tile_pool(name="io", bufs=2) as io,
            tc.tile_pool(name="tmp", bufs=2) as tmp,
        ):
            lcol_t = const.tile([P, N], f32)
            nc.sync.dma_start(lcol_t[:], lcol[:, :])
            lrow_t = const.tile([P, RT], f32)
            nc.sync.dma_start(lrow_t[:], lrow[:, :])

            for rt in range(RT):
                rs = slice(rt * P, (rt + 1) * P)
                for c in range(CT):
                    cs = slice(c * W, (c + 1) * W)
                    sd_t = io.tile([P, W], f32, tag="sd")
                    nc.sync.dma_start(sd_t[:], sd[rs, cs])
                    dtw_t = io.tile([P, W], f32, tag="dtw")
                    nc.sync.dma_start(dtw_t[:], dtw[rs, cs])

                    z1_t = tmp.tile([P, W], f32, tag="z1")
                    nc.scalar.activation(z1_t[:], sd_t[:], AF.Square)
                    z2_t = tmp.tile([P, W], f32, tag="z2")
                    nc.scalar.activation(z2_t[:], dtw_t[:], AF.Square)
                    s_t = tmp.tile([P, W], f32, tag="s")
                    nc.vector.tensor_add(s_t[:], z1_t[:], z2_t[:])
                    e_t = tmp.tile([P, W], f32, tag="e")
                    nc.scalar.activation(e_t[:], s_t[:], AF.Exp, scale=-1.0)

                    aext_t = tmp.tile([P, W], f32, tag="aext")
                    nc.vector.tensor_scalar(
                        aext_t[:], lcol_t[:, cs], lrow_t[:, rt:rt + 1], 0.1,
                        op0=OP.is_equal, op1=OP.max,
                    )
                    me_t = tmp.tile([P, W], f32, tag="me")
                    nc.vector.scalar_tensor_tensor(
                        me_t[:], sd_t[:], 5.0, e_t[:],
                        op0=OP.is_lt, op1=OP.mult,
                    )
                    out_t = io.tile([P, W], f32, tag="out")
                    nc.vector.tensor_mul(out_t[:], me_t[:], aext_t[:])
                    nc.sync.dma_start(out[rs, cs], out_t[:])

    nc.compile()
    return nc


def kernel(adj_mx, sd_mx, dtw_matrix, cluster_labels):
    from concourse.bass_utils import run_bass_kernel_spmd

    if "nc" not in _CACHE:
        _CACHE["nc"] = _build()
    nc = _CACHE["nc"]

    sd_mx = np.asarray(sd_mx, dtype=np.float32)
    dtw_matrix = np.asarray(dtw_matrix, dtype=np.float32)
    labels_f32 = np.asarray(cluster_labels).astype(np.float32)

    lcol = np.ascontiguousarray(np.broadcast_to(labels_f32[None, :], (P, N)))
    in_maps = []
    for core in range(N_CORES):
        r0 = core * R
        lrow = np.ascontiguousarray(
            labels_f32[r0:r0 + R].reshape(RT, P).T)
        in_maps.append({
            "sd": np.ascontiguousarray(sd_mx[r0:r0 + R]),
            "dtw": np.ascontiguousarray(dtw_matrix[r0:r0 + R]),
            "lcol": lcol,
            "lrow": lrow,
        })

    res = run_bass_kernel_spmd(nc, in_maps, list(range(N_CORES)))
    return np.concatenate([res.results[i]["out"] for i in range(N_CORES)],
                          axis=0)



# revision 7
# speedup vs baseline: 957.7705x; 957.7705x over previous
"""Trainium2 Bass kernel for the DTW mask calculator.

Computes, for N=8192, fp32 inputs:
    out = where(sd < 5, exp(-sd^2), 0) * where(labels[i]==labels[j], 1, 0.1)
          * exp(-dtw^2)

Row-sharded across 8 NeuronCores (1024 rows each). adj_mx is unused by the
reference computation and never uploaded.

Implementation notes:
- Inputs are staged to HBM as fp16 (tolerance is 2e-2; measured pipeline
  error is ~5e-4). This halves HBM traffic, the binding resource.
- The sd<5 gate falls out of fp16 underflow: sd>=5 => sd^2+dtw^2 >= 25 =>
  exp(-25) ~ 1.4e-11 rounds to 0 in fp16 (min subnormal 6e-8). Elements
  with sd slightly below 5 differ from the reference by <2e-8 absolute.
- The 0.1 cluster factor is folded into the exponent:
      out = exp(-(sd^2 + dtw^2 + ln(10)*(l_i != l_j)))
  The per-row-tile penalty plane is built once per 128-row tile on GPSIMD,
  off the DVE critical path.
- Per [128, 4096] fp16 tile: DVE does sd*sd and two adds (3 tensor_tensor
  at the 2x 16-bit rate ~ 6.9us), ACT does Square(dtw) and Exp (both in
  the exp_and_others table set, ~7.4us), DMA moves 3MB (~8.4us at
  358 GB/s) -- DMA-bound as intended for this memory-regime problem.

``_build(reps=K)`` unrolls the whole per-core computation K times inside
one NEFF; test.py uses (T(reps=K) - T(reps=1)) / (K-1) to measure the
per-iteration hardware time below the ~70ms axon dispatch floor.
"""

import numpy as np

N = 8192
N_CORES = 8
R = N // N_CORES          # rows per core = 1024
P = 128                   # partitions
RT = R // P               # row tiles per core = 8
W = 4096                  # column chunk width
CT = N // W               # column chunks = 2
LN10 = 2.302585092994046  # exp(-LN10) == 0.1

_CACHE = {}


def _build(reps=1):
    import concourse.tile as tile
    from concourse import bacc, mybir

    f16 = mybir.dt.float16
    f32 = mybir.dt.float32
    AF = mybir.ActivationFunctionType
    OP = mybir.AluOpType

    nc = bacc.Bacc("TRN2", target_bir_lowering=False, debug=False,
                   num_devices=N_CORES)

    sd = nc.dram_tensor("sd", [R, N], f16, kind="ExternalInput").ap()
    dtw = nc.dram_tensor("dtw", [R, N], f16, kind="ExternalInput").ap()
    lcol = nc.dram_tensor("lcol", [P, N], f16, kind="ExternalInput").ap()
    lrow = nc.dram_tensor("lrow", [P, RT], f32, kind="ExternalInput").ap()
    out = nc.dram_tensor("out", [R, N], f16, kind="ExternalOutput").ap()

    with tile.TileContext(nc) as tc:
        with (
            tc.tile_pool(name="const", bufs=1) as const,
            tc.tile_pool(name="pen", bufs=2) as pen,
            tc.tile_pool(name="io", bufs=2) as io,
            tc.tile_pool(name="tmp", bufs=2) as tmp,
        ):
            lcol_t = const.tile([P, N], f16)
            nc.sync.dma_start(lcol_t[:], lcol[:, :])
            lrow_t = const.tile([P, RT], f32)
            nc.sync.dma_start(lrow_t[:], lrow[:, :])

            for rep in range(reps):
                for rt in range(RT):
                    rs = slice(rt * P, (rt + 1) * P)
                    # penalty plane: ln(10) where labels differ, else 0
                    pen_t = pen.tile([P, N], f16, tag="pen")
                    nc.vector.tensor_scalar(
                        pen_t[:], lcol_t[:], lrow_t[:, rt:rt + 1], LN10,
                        op0=OP.not_equal, op1=OP.mult,
                    )
                    for c in range(CT):
                        cs = slice(c * W, (c + 1) * W)
                        sd_t = io.tile([P, W], f16, tag="sd")
                        nc.sync.dma_start(sd_t[:], sd[rs, cs])
                        dtw_t = io.tile([P, W], f16, tag="dtw")
                        nc.sync.dma_start(dtw_t[:], dtw[rs, cs])

                        z2_t = tmp.tile([P, W], f16, tag="z2")
                        nc.scalar.activation(z2_t[:], dtw_t[:], AF.Square)
                        z1_t = tmp.tile([P, W], f16, tag="z1")
                        nc.vector.tensor_mul(z1_t[:], sd_t[:], sd_t[:])
                        s_t = tmp.tile([P, W], f16, tag="s")
                        nc.vector.tensor_add(s_t[:], z1_t[:], z2_t[:])
                        st_t = tmp.tile([P, W], f16, tag="st")
                        nc.vector.tensor_add(st_t[:], s_t[:], pen_t[:, cs])

                        out_t = io.tile([P, W], f16, tag="out")
                        nc.scalar.activation(out_t[:], st_t[:], AF.Exp,
                                             scale=-1.0)
                        nc.sync.dma_start(out[rs, cs], out_t[:])

    nc.compile()
    return nc


def kernel(adj_mx, sd_mx, dtw_matrix, cluster_labels):
    from concourse.bass_utils import run_bass_kernel_spmd

    if "nc" not in _CACHE:
        _CACHE["nc"] = _build()
    nc = _CACHE["nc"]

    sd16 = np.asarray(sd_mx, dtype=np.float32).astype(np.float16)
    dtw16 = np.asarray(dtw_matrix, dtype=np.float32).astype(np.float16)
    labels16 = np.asarray(cluster_labels).astype(np.float16)

    lcol = np.ascontiguousarray(np.broadcast_to(labels16[None, :], (P, N)))
    in_maps = []
    for core in range(N_CORES):
        r0 = core * R
        lrow = np.ascontiguousarray(
            labels16[r0:r0 + R].reshape(RT, P).T.astype(np.float32))
        in_maps.append({
            "sd": np.ascontiguousarray(sd16[r0:r0 + R]),
            "dtw": np.ascontiguousarray(dtw16[r0:r0 + R]),
            "lcol": lcol,
            "lrow": lrow,
        })

    res = run_bass_kernel_spmd(nc, in_maps, list(range(N_CORES)))
    out16 = np.concatenate([res.results[i]["out"] for i in range(N_CORES)],
                           axis=0)
    return out16.astype(np.float32)
